# revision 28
# baseline (speedup 1.0000x reference)
"""v14 experiment: 2KB DMA lines via host-side k-slice-pair packing.

Identical pipeline to v13, but the host packs TWO k-slices per DRAM row
(row (c*512 + q*128 + p) = [slice 2q row p | slice 2q+1 row p], each
512 bf16 cols) so every load DMA moves 256KB as 128 descriptors of 2KB
contiguous runs. Per-packet service is ~68ns/1KB vs ~105ns/2KB, so 1KB
lines waste ~23% of fabric; this restores full-rate loads. The matmuls
address the packed SBUF tiles: slice k lives in cols (k%2)*512 of
packed tile q=k//2. 16 load DMAs instead of 32; one pair sem per
packed tile (w DMA + x DMA, waited at full 32).
"""

import numpy as np
import ml_dtypes

import concourse.bass as bass
import concourse.mybir as mybir
from concourse.bass_utils import run_bass_kernel_spmd

B, IN_F, OUT_F = 8192, 1024, 1024
N_CORES = 8
M = B // N_CORES
P = 128
MB = 512
KT = IN_F // P    # 8
NT = OUT_F // P   # 8
CB = 512
QT = KT // 2      # packed tiles per column-block (4)
NGROUPS = (M // MB) * NT
NWARM = 48
WROWS = 64

F32 = mybir.dt.float32
BF16 = mybir.dt.bfloat16


def build_program() -> bass.Bass:
    nc = bass.Bass()
    # Packed layouts: row (c*512 + q*128 + p), cols [slice 2q | slice 2q+1]
    xTp = nc.declare_dram_parameter("xTp", [IN_F, M], BF16, isOutput=False)
    wTp = nc.declare_dram_parameter("wTp", [IN_F, OUT_F], BF16, isOutput=False)
    bias = nc.declare_dram_parameter("bias", [P, NT], F32, isOutput=False)
    outT = nc.declare_dram_parameter("outT", [OUT_F, M], BF16, isOutput=True)

    import contextlib

    with contextlib.ExitStack() as ctx:
        # Packed SBUF tiles: [c][q] holds slices k=2q (cols 0:512) and
        # k=2q+1 (cols 512:1024).
        wt_sb = [
            [ctx.enter_context(nc.sbuf_tensor(f"wt{c}_{q}", [P, 2 * CB], BF16))
             for q in range(QT)]
            for c in range(2)
        ]
        xt_sb = [
            [ctx.enter_context(nc.sbuf_tensor(f"xt{c}_{q}", [P, 2 * CB], BF16))
             for q in range(QT)]
            for c in range(2)
        ]
        ot_sb = [
            ctx.enter_context(nc.sbuf_tensor(f"ot{j}", [P, MB], BF16))
            for j in range(4)
        ]
        bias_sb = ctx.enter_context(nc.sbuf_tensor("bias_sb", [P, NT], F32))
        warm_sb = ctx.enter_context(nc.sbuf_tensor("warm_sb", [P, P], BF16))
        ps = [
            ctx.enter_context(nc.psum_tensor(f"ps{b}", [P, MB], F32))
            for b in range(8)
        ]
        ld_b = ctx.enter_context(nc.semaphore("ld_b"))
        warm = ctx.enter_context(nc.semaphore("warm"))
        mm = ctx.enter_context(nc.semaphore("mm"))
        ev = ctx.enter_context(nc.semaphore("ev"))
        ev_h0 = ctx.enter_context(nc.semaphore("ev_h0"))
        pair = [
            [ctx.enter_context(nc.semaphore(f"pair{c}_{q}"))
             for q in range(QT)]
            for c in range(2)
        ]
        st_s = [
            ctx.enter_context(nc.semaphore(f"st{j}")) for j in range(4)
        ]

        def wtile(c, k, j):  # stationary [128,128] tile (slice k, nt j)
            off = (k % 2) * CB + j * P
            return wt_sb[c][k // 2][:, off:off + P]

        def xslice(c, k):  # moving operand for slice k, m-block c
            return xt_sb[c][k // 2][:, (k % 2) * CB:(k % 2) * CB + CB]

        with nc.Block(no_gpsimd_drain=True) as block:

            @block.sync
            def _(sync):
                for c in range(2):
                    for q in range(QT):
                        sync.dma_start(
                            out=wt_sb[c][q][:],
                            in_=wTp[c * 4 * P + q * P:c * 4 * P + (q + 1) * P, :],
                        ).then_inc(pair[c][q], 16)

            @block.gpsimd
            def _(gpsimd):
                gpsimd.memset(warm_sb[:], 0).then_inc(warm, 1)
                gpsimd.dma_start(out=bias_sb[:], in_=bias[:]).then_inc(ld_b, 16)

            @block.scalar
            def _(scalar):
                for c in range(2):
                    for q in range(QT):
                        scalar.dma_start(
                            out=xt_sb[c][q][:],
                            in_=xTp[c * 4 * P + q * P:c * 4 * P + (q + 1) * P, :],
                        ).then_inc(pair[c][q], 16)
                for g in range(NGROUPS - 1):
                    mb, nt = divmod(g, NT)
                    scalar.wait_ge(ev, g + 1)
                    scalar.dma_start(
                        out=outT[nt * P:(nt + 1) * P, mb * MB:(mb + 1) * MB],
                        in_=ot_sb[g % 4][:],
                    ).then_inc(st_s[g % 4], 16)
                scalar.wait_ge(ev_h0, 1)
                scalar.dma_start(
                    out=outT[7 * P:8 * P, MB:2 * MB],
                    in_=ot_sb[3][:],
                ).then_inc(st_s[3], 16)
                for j in range(3):
                    scalar.wait_ge(st_s[j], (NGROUPS // 4) * 16)
                scalar.wait_ge(st_s[3], 4 * 16)

            @block.tensor
            def _(tensor):
                tensor.wait_ge(warm, 1)
                for _ in range(NWARM - 3):
                    tensor.matmul(
                        ps[7][:, 0:WROWS],
                        warm_sb[:, :],
                        warm_sb[:, 0:WROWS],
                        start=True,
                        stop=True,
                    )
                tensor.wait_ge(pair[0][0], 32)
                for _ in range(3):
                    tensor.matmul(
                        ps[7][:, 0:WROWS],
                        warm_sb[:, :],
                        warm_sb[:, 0:WROWS],
                        start=True,
                        stop=True,
                    )
                for phase in range(3):
                    mb = phase // 2
                    cw = phase % 2
                    bank0 = cw * 4
                    if phase == 2:
                        tensor.wait_ge(ev, 4)
                    for k in range(KT):
                        if phase == 0 and k % 2 == 0:
                            tensor.wait_ge(pair[0][k // 2], 32)
                        elif phase == 1 and k % 2 == 0:
                            tensor.wait_ge(pair[1][k // 2], 32)
                        for j in range(4):
                            inst = tensor.matmul(
                                ps[bank0 + j][:, :],
                                wtile(cw, k, j),
                                xslice(mb, k),
                                start=(k == 0),
                                stop=(k == KT - 1),
                            )
                            if k == KT - 1:
                                inst.then_inc(mm, 1)
                tensor.wait_ge(ev, 8)
                for g in range(12, NGROUPS):
                    nt = g - 8
                    ni = nt - 4
                    inst = None
                    for k in range(KT):
                        inst = tensor.matmul(
                            ps[4 + ni][:, :],
                            wtile(1, k, ni),
                            xslice(1, k),
                            start=(k == 0),
                            stop=(k == KT - 1),
                        )
                    inst.then_inc(mm, 1)

            @block.vector
            def _(vector):
                vector.wait_ge(ld_b, 16)
                for g in range(NGROUPS - 1):
                    mb, nt = divmod(g, NT)
                    vector.wait_ge(mm, g + 1)
                    if g >= 4:
                        vector.wait_ge(st_s[g % 4], (g // 4) * 16)
                    vector.tensor_scalar_add(
                        ot_sb[g % 4][:],
                        ps[g % 8][:, :],
                        bias_sb[:, nt:nt + 1],
                    ).then_inc(ev, 1)
                vector.wait_ge(mm, NGROUPS)
                vector.wait_ge(st_s[3], 48)
                vector.tensor_scalar_add(
                    ot_sb[3][:],
                    ps[7][:, :],
                    bias_sb[:, 7:8],
                ).then_inc(ev_h0, 1)

    return nc


_PROGRAM = None


def _get_program() -> bass.Bass:
    global _PROGRAM
    if _PROGRAM is None:
        _PROGRAM = build_program()
    return _PROGRAM


def _pack(t: np.ndarray) -> np.ndarray:
    """[1024, 1024] k-major -> packed: row (c*512+q*128+p) =
    [slice 2q row p cols c | slice 2q+1 row p cols c]."""
    a = t.reshape(4, 2, P, 2, CB)          # (q, kp, p, c, j)
    return np.ascontiguousarray(
        a.transpose(3, 0, 2, 1, 4).reshape(IN_F, 2 * CB)
    )


def make_in_maps(x: np.ndarray, W: np.ndarray, b: np.ndarray) -> list[dict]:
    WTp = _pack(W.T.astype(ml_dtypes.bfloat16))
    bias = np.ascontiguousarray(
        b.astype(np.float32, copy=False).reshape(NT, P).T
    )
    in_maps = []
    for c in range(N_CORES):
        xTp = _pack(x[c * M:(c + 1) * M, :].T.astype(ml_dtypes.bfloat16))
        in_maps.append({"xTp": xTp, "wTp": WTp, "bias": bias})
    return in_maps


def assemble_output(results: list[dict]) -> np.ndarray:
    out = np.empty((B, OUT_F), dtype=np.float32)
    for c in range(N_CORES):
        out[c * M:(c + 1) * M, :] = results[c]["outT"].T.astype(np.float32)
    return out


def kernel(x: np.ndarray, W: np.ndarray, b: np.ndarray) -> np.ndarray:
    nc = _get_program()
    in_maps = make_in_maps(np.asarray(x), np.asarray(W), np.asarray(b))
    res = run_bass_kernel_spmd(nc, in_maps, list(range(N_CORES)))
    return assemble_output(res.results)


# revision 30
# speedup vs baseline: 1.1408x; 1.1408x over previous
"""v15: hybrid of v13/v14 granularities on the v14 packed layout.

The first packed c0 tile loads as TWO column-half DMAs (slice k0 then
k1, 128KB each, 1KB lines) so the first pair lands as early as v13's,
while the remaining 3.5MB load as full 256KB packed DMAs with 2KB
lines at the ~17%-higher fabric rate v14 measured. Original v14 note:

Identical pipeline to v13, but the host packs TWO k-slices per DRAM row
(row (c*512 + q*128 + p) = [slice 2q row p | slice 2q+1 row p], each
512 bf16 cols) so every load DMA moves 256KB as 128 descriptors of 2KB
contiguous runs. Per-packet service is ~68ns/1KB vs ~105ns/2KB, so 1KB
lines waste ~23% of fabric; this restores full-rate loads. The matmuls
address the packed SBUF tiles: slice k lives in cols (k%2)*512 of
packed tile q=k//2. 16 load DMAs instead of 32; one pair sem per
packed tile (w DMA + x DMA, waited at full 32).
"""

import numpy as np
import ml_dtypes

import concourse.bass as bass
import concourse.mybir as mybir
from concourse.bass_utils import run_bass_kernel_spmd

B, IN_F, OUT_F = 8192, 1024, 1024
N_CORES = 8
M = B // N_CORES
P = 128
MB = 512
KT = IN_F // P    # 8
NT = OUT_F // P   # 8
CB = 512
QT = KT // 2      # packed tiles per column-block (4)
NGROUPS = (M // MB) * NT
NWARM = 48
WROWS = 64

F32 = mybir.dt.float32
BF16 = mybir.dt.bfloat16


def build_program() -> bass.Bass:
    nc = bass.Bass()
    # Packed layouts: row (c*512 + q*128 + p), cols [slice 2q | slice 2q+1]
    xTp = nc.declare_dram_parameter("xTp", [IN_F, M], BF16, isOutput=False)
    wTp = nc.declare_dram_parameter("wTp", [IN_F, OUT_F], BF16, isOutput=False)
    bias = nc.declare_dram_parameter("bias", [P, NT], F32, isOutput=False)
    outT = nc.declare_dram_parameter("outT", [OUT_F, M], BF16, isOutput=True)

    import contextlib

    with contextlib.ExitStack() as ctx:
        # Packed SBUF tiles: [c][q] holds slices k=2q (cols 0:512) and
        # k=2q+1 (cols 512:1024).
        wt_sb = [
            [ctx.enter_context(nc.sbuf_tensor(f"wt{c}_{q}", [P, 2 * CB], BF16))
             for q in range(QT)]
            for c in range(2)
        ]
        xt_sb = [
            [ctx.enter_context(nc.sbuf_tensor(f"xt{c}_{q}", [P, 2 * CB], BF16))
             for q in range(QT)]
            for c in range(2)
        ]
        ot_sb = [
            ctx.enter_context(nc.sbuf_tensor(f"ot{j}", [P, MB], BF16))
            for j in range(4)
        ]
        bias_sb = ctx.enter_context(nc.sbuf_tensor("bias_sb", [P, NT], F32))
        warm_sb = ctx.enter_context(nc.sbuf_tensor("warm_sb", [P, P], BF16))
        ps = [
            ctx.enter_context(nc.psum_tensor(f"ps{b}", [P, MB], F32))
            for b in range(8)
        ]
        ld_b = ctx.enter_context(nc.semaphore("ld_b"))
        warm = ctx.enter_context(nc.semaphore("warm"))
        mm = ctx.enter_context(nc.semaphore("mm"))
        ev = ctx.enter_context(nc.semaphore("ev"))
        ev_h0 = ctx.enter_context(nc.semaphore("ev_h0"))
        pair0b = ctx.enter_context(nc.semaphore("pair0b"))  # k1 half-tile
        pair = [
            [ctx.enter_context(nc.semaphore(f"pair{c}_{q}"))
             for q in range(QT)]
            for c in range(2)
        ]
        st_s = [
            ctx.enter_context(nc.semaphore(f"st{j}")) for j in range(4)
        ]

        def wtile(c, k, j):  # stationary [128,128] tile (slice k, nt j)
            off = (k % 2) * CB + j * P
            return wt_sb[c][k // 2][:, off:off + P]

        def xslice(c, k):  # moving operand for slice k, m-block c
            return xt_sb[c][k // 2][:, (k % 2) * CB:(k % 2) * CB + CB]

        with nc.Block(no_gpsimd_drain=True) as block:

            @block.sync
            def _(sync):
                # First tile split by column halves: slice k0 (cols
                # 0:512) arrives a full 256KB earlier than a packed
                # load would deliver it; the rest use 2KB-line DMAs.
                sync.dma_start(
                    out=wt_sb[0][0][:, 0:CB],
                    in_=wTp[0:P, 0:CB],
                ).then_inc(pair[0][0], 16)
                sync.dma_start(
                    out=wt_sb[0][0][:, CB:2 * CB],
                    in_=wTp[0:P, CB:2 * CB],
                ).then_inc(pair0b, 16)
                for c in range(2):
                    for q in range(QT):
                        if c == 0 and q == 0:
                            continue
                        sync.dma_start(
                            out=wt_sb[c][q][:],
                            in_=wTp[c * 4 * P + q * P:c * 4 * P + (q + 1) * P, :],
                        ).then_inc(pair[c][q], 16)

            @block.gpsimd
            def _(gpsimd):
                gpsimd.memset(warm_sb[:], 0).then_inc(warm, 1)
                gpsimd.dma_start(out=bias_sb[:], in_=bias[:]).then_inc(ld_b, 16)

            @block.scalar
            def _(scalar):
                scalar.dma_start(
                    out=xt_sb[0][0][:, 0:CB],
                    in_=xTp[0:P, 0:CB],
                ).then_inc(pair[0][0], 16)
                scalar.dma_start(
                    out=xt_sb[0][0][:, CB:2 * CB],
                    in_=xTp[0:P, CB:2 * CB],
                ).then_inc(pair0b, 16)
                for c in range(2):
                    for q in range(QT):
                        if c == 0 and q == 0:
                            continue
                        scalar.dma_start(
                            out=xt_sb[c][q][:],
                            in_=xTp[c * 4 * P + q * P:c * 4 * P + (q + 1) * P, :],
                        ).then_inc(pair[c][q], 16)
                for g in range(NGROUPS - 1):
                    mb, nt = divmod(g, NT)
                    scalar.wait_ge(ev, g + 1)
                    scalar.dma_start(
                        out=outT[nt * P:(nt + 1) * P, mb * MB:(mb + 1) * MB],
                        in_=ot_sb[g % 4][:],
                    ).then_inc(st_s[g % 4], 16)
                scalar.wait_ge(ev_h0, 1)
                scalar.dma_start(
                    out=outT[7 * P:8 * P, MB:2 * MB],
                    in_=ot_sb[3][:],
                ).then_inc(st_s[3], 16)
                for j in range(3):
                    scalar.wait_ge(st_s[j], (NGROUPS // 4) * 16)
                scalar.wait_ge(st_s[3], 4 * 16)

            @block.tensor
            def _(tensor):
                tensor.wait_ge(warm, 1)
                for _ in range(NWARM - 3):
                    tensor.matmul(
                        ps[7][:, 0:WROWS],
                        warm_sb[:, :],
                        warm_sb[:, 0:WROWS],
                        start=True,
                        stop=True,
                    )
                tensor.wait_ge(pair[0][0], 32)
                for _ in range(3):
                    tensor.matmul(
                        ps[7][:, 0:WROWS],
                        warm_sb[:, :],
                        warm_sb[:, 0:WROWS],
                        start=True,
                        stop=True,
                    )
                for phase in range(3):
                    mb = phase // 2
                    cw = phase % 2
                    bank0 = cw * 4
                    if phase == 2:
                        tensor.wait_ge(ev, 4)
                    for k in range(KT):
                        if phase == 0 and k == 1:
                            tensor.wait_ge(pair0b, 32)
                        elif phase == 0 and k % 2 == 0:
                            tensor.wait_ge(pair[0][k // 2], 32)
                        elif phase == 1 and k % 2 == 0:
                            tensor.wait_ge(pair[1][k // 2], 32)
                        for j in range(4):
                            inst = tensor.matmul(
                                ps[bank0 + j][:, :],
                                wtile(cw, k, j),
                                xslice(mb, k),
                                start=(k == 0),
                                stop=(k == KT - 1),
                            )
                            if k == KT - 1:
                                inst.then_inc(mm, 1)
                tensor.wait_ge(ev, 8)
                for g in range(12, NGROUPS):
                    nt = g - 8
                    ni = nt - 4
                    inst = None
                    for k in range(KT):
                        inst = tensor.matmul(
                            ps[4 + ni][:, :],
                            wtile(1, k, ni),
                            xslice(1, k),
                            start=(k == 0),
                            stop=(k == KT - 1),
                        )
                    inst.then_inc(mm, 1)

            @block.vector
            def _(vector):
                vector.wait_ge(ld_b, 16)
                for g in range(NGROUPS - 1):
                    mb, nt = divmod(g, NT)
                    vector.wait_ge(mm, g + 1)
                    if g >= 4:
                        vector.wait_ge(st_s[g % 4], (g // 4) * 16)
                    vector.tensor_scalar_add(
                        ot_sb[g % 4][:],
                        ps[g % 8][:, :],
                        bias_sb[:, nt:nt + 1],
                    ).then_inc(ev, 1)
                vector.wait_ge(mm, NGROUPS)
                vector.wait_ge(st_s[3], 48)
                vector.tensor_scalar_add(
                    ot_sb[3][:],
                    ps[7][:, :],
                    bias_sb[:, 7:8],
                ).then_inc(ev_h0, 1)

    return nc


_PROGRAM = None


def _get_program() -> bass.Bass:
    global _PROGRAM
    if _PROGRAM is None:
        _PROGRAM = build_program()
    return _PROGRAM


def _pack(t: np.ndarray) -> np.ndarray:
    """[1024, 1024] k-major -> packed: row (c*512+q*128+p) =
    [slice 2q row p cols c | slice 2q+1 row p cols c]."""
    a = t.reshape(4, 2, P, 2, CB)          # (q, kp, p, c, j)
    return np.ascontiguousarray(
        a.transpose(3, 0, 2, 1, 4).reshape(IN_F, 2 * CB)
    )


def make_in_maps(x: np.ndarray, W: np.ndarray, b: np.ndarray) -> list[dict]:
    WTp = _pack(W.T.astype(ml_dtypes.bfloat16))
    bias = np.ascontiguousarray(
        b.astype(np.float32, copy=False).reshape(NT, P).T
    )
    in_maps = []
    for c in range(N_CORES):
        xTp = _pack(x[c * M:(c + 1) * M, :].T.astype(ml_dtypes.bfloat16))
        in_maps.append({"xTp": xTp, "wTp": WTp, "bias": bias})
    return in_maps


def assemble_output(results: list[dict]) -> np.ndarray:
    out = np.empty((B, OUT_F), dtype=np.float32)
    for c in range(N_CORES):
        out[c * M:(c + 1) * M, :] = results[c]["outT"].T.astype(np.float32)
    return out


def kernel(x: np.ndarray, W: np.ndarray, b: np.ndarray) -> np.ndarray:
    nc = _get_program()
    in_maps = make_in_maps(np.asarray(x), np.asarray(W), np.asarray(b))
    res = run_bass_kernel_spmd(nc, in_maps, list(range(N_CORES)))
    return assemble_output(res.results)
